# revision 9
# baseline (speedup 1.0000x reference)
"""GraphSAGE (2x SAGEConv mean-aggr + linear head + log_softmax) on 8 trn2 cores.

Sharding: nodes (and their incoming edges) are partitioned across 8 cores by
dst node.  Each core:
  - gathers x[src] rows for its edges from a replicated copy of x in DRAM
    (indirect DMA, 128-row tiles),
  - segment-sums them into per-dst-window aggregates with PSUM-accumulated
    matmuls against a device-built one-hot selection matrix,
  - scales by 1/deg (host precomputed) and applies the dense SAGE transforms
    as feature-major matmuls (weights pre-transposed on host),
  - AllGathers h1 across cores, repeats for layer 2, then computes the
    2-class log_softmax head.
"""

import numpy as np

N_NODES = 50000
N_FEAT = 128
N_CORES = 8
NL = N_NODES // N_CORES          # 6250 nodes per core
P = 128                          # partitions / window size
NW = (NL + P - 1) // P           # 49 dst windows per core
NLP = NW * P                     # 6272 padded local nodes
F32 = np.float32


# ---------------------------------------------------------------- host prep

def _prep_graph(edge_index):
    """Partition + sort edges by dst, build per-core per-window padded edge
    tiles.  Returns per-core idx (layer1 / layer2), dstoff, rdeg arrays and
    the common per-window tile counts."""
    src = np.asarray(edge_index[0], dtype=np.int64)
    dst = np.asarray(edge_index[1], dtype=np.int64)

    deg = np.bincount(dst, minlength=N_NODES).astype(F32)
    rdeg = 1.0 / np.maximum(deg, 1.0)

    core_of = dst // NL
    per_core = []
    for c in range(N_CORES):
        sel = core_of == c
        s_c = src[sel]
        d_c = dst[sel] - c * NL
        order = np.argsort(d_c, kind="stable")
        per_core.append((s_c[order], d_c[order]))

    # per-window edge counts, common tile count across cores (SPMD program)
    counts = np.zeros((N_CORES, NW), dtype=np.int64)
    for c in range(N_CORES):
        _, d_c = per_core[c]
        w = d_c // P
        counts[c] = np.bincount(w, minlength=NW)
    tiles_per_w = np.maximum((counts.max(axis=0) + P - 1) // P, 1).astype(np.int64)
    t_total = int(tiles_per_w.sum())

    idx1 = np.zeros((N_CORES, 128, t_total), dtype=np.int32)
    idx2 = np.zeros((N_CORES, 128, t_total), dtype=np.int32)
    dstoff = np.zeros((N_CORES, 128, t_total), dtype=F32)
    for c in range(N_CORES):
        s_c, d_c = per_core[c]
        w_c = d_c // P
        starts = np.concatenate([[0], np.cumsum(counts[c])])
        i1 = np.zeros(t_total * 128, dtype=np.int32)
        doff = np.full(t_total * 128, -1.0, dtype=F32)
        pos = 0
        for w in range(NW):
            n = int(counts[c][w])
            cap = int(tiles_per_w[w]) * 128
            sl = slice(starts[w], starts[w] + n)
            i1[pos:pos + n] = s_c[sl]
            doff[pos:pos + n] = (d_c[sl] - w * P).astype(F32)
            pos += cap
        # edge e (= t*128 + p) -> [p, t]
        i1 = i1.reshape(t_total, 128).T
        idx1[c] = np.ascontiguousarray(i1)
        i2 = (i1 // NL) * NLP + (i1 % NL)        # node id in allgathered h1
        idx2[c] = np.ascontiguousarray(i2.astype(np.int32))
        dstoff[c] = np.ascontiguousarray(doff.reshape(t_total, 128).T)

    rdeg_core = np.ones((N_CORES, 128, NLP), dtype=F32)
    for c in range(N_CORES):
        r = np.ones(NLP, dtype=F32)
        r[:NL] = rdeg[c * NL:(c + 1) * NL]
        rdeg_core[c] = np.broadcast_to(r, (128, NLP))

    return idx1, idx2, dstoff, rdeg_core, tiles_per_w


# ---------------------------------------------------------------- device build

def _build_nc(tiles_per_w, n_nodes=N_NODES, nl=NL, nw=NW, n_cores=N_CORES):
    import concourse.bass as bass
    import concourse.mybir as mybir
    from concourse import bacc
    from concourse.tile import TileContext

    nlp = nw * P
    t_total = int(np.sum(tiles_per_w))
    dt = mybir.dt

    nc = bacc.Bacc(None, target_bir_lowering=False, debug=False,
                   num_devices=n_cores)

    # -------- I/O
    x_full = nc.dram_tensor("x_full", [n_nodes, N_FEAT], dt.float32,
                            kind="ExternalInput")
    xT = nc.dram_tensor("xT", [128, nlp], dt.float32, kind="ExternalInput")
    rdeg = nc.dram_tensor("rdeg", [128, nlp], dt.float32, kind="ExternalInput")
    idx1 = nc.dram_tensor("idx1", [128, t_total], dt.int32, kind="ExternalInput")
    idx2 = nc.dram_tensor("idx2", [128, t_total], dt.int32, kind="ExternalInput")
    dstoff = nc.dram_tensor("dstoff", [128, t_total], dt.float32,
                            kind="ExternalInput")
    iota_in = nc.dram_tensor("iota", [128, 128], dt.float32, kind="ExternalInput")
    ident_in = nc.dram_tensor("ident", [128, 128], dt.float32,
                              kind="ExternalInput")
    w_ins = {}
    for nm in ("W1lT", "W1rT", "W2lT", "W2rT"):
        w_ins[nm] = nc.dram_tensor(nm, [128, 128], dt.float32,
                                   kind="ExternalInput")
    wlinT_in = nc.dram_tensor("WlinT", [128, 2], dt.float32, kind="ExternalInput")
    b1l_in = nc.dram_tensor("b1l", [128, 1], dt.float32, kind="ExternalInput")
    b2l_in = nc.dram_tensor("b2l", [128, 1], dt.float32, kind="ExternalInput")
    blin_in = nc.dram_tensor("blin", [128, 2], dt.float32, kind="ExternalInput")
    out_d = nc.dram_tensor("out", [nlp, 2], dt.float32, kind="ExternalOutput")

    # -------- internal DRAM
    h1_local = nc.dram_tensor("h1_local", [nlp, 128], dt.float32)
    h1_full = nc.dram_tensor("h1_full", [n_cores * nlp, 128], dt.float32)

    def bcast_mid(ap2d, t):
        # [P, X] -> [P, t, X] (0-step middle dim)
        return bass.AP(ap2d.tensor, ap2d.offset, [ap2d.ap[0], [0, t], ap2d.ap[1]])

    def bcast_inner(ap2d, x):
        # [P, T] -> [P, T, x] (0-step inner dim)
        return bass.AP(ap2d.tensor, ap2d.offset, [ap2d.ap[0], ap2d.ap[1], [0, x]])

    with TileContext(nc) as tc:
        with (
            tc.tile_pool(name="const", bufs=1) as cpool,
            tc.tile_pool(name="big", bufs=1) as bpool,
            tc.tile_pool(name="work", bufs=3) as wpool,
            tc.tile_pool(name="psum", bufs=1, space="PSUM") as ppool,
        ):
            # resident constants
            def load_const(src_ap, shape, dtype=dt.float32, name=None):
                t = cpool.tile(shape, dtype, name=name)
                nc.sync.dma_start(t[:], src_ap)
                return t

            iota_sb = load_const(iota_in[:, :], [128, 128], name="iota_sb")
            ident_sb = load_const(ident_in[:, :], [128, 128], name="ident_sb")
            w_sb = {nm: load_const(w_ins[nm][:, :], [128, 128], name=nm + "_sb")
                    for nm in w_ins}
            wlinT_sb = load_const(wlinT_in[:, :], [128, 2], name="wlinT_sb")
            b1l_sb = load_const(b1l_in[:, :], [128, 1], name="b1l_sb")
            b2l_sb = load_const(b2l_in[:, :], [128, 1], name="b2l_sb")
            blin_sb = load_const(blin_in[:, :], [128, 2], name="blin_sb")
            idx1_sb = load_const(idx1[:, :], [128, t_total], dt.int32, "idx1_sb")
            idx2_sb = load_const(idx2[:, :], [128, t_total], dt.int32, "idx2_sb")
            dstoff_sb = load_const(dstoff[:, :], [128, t_total], name="dstoff_sb")
            rdeg_sb = load_const(rdeg[:, :], [128, nlp], name="rdeg_sb")
            xT_sb = load_const(xT[:, :], [128, nlp], name="xT_sb")

            h1T_sb = bpool.tile([128, nlp], dt.float32, name="h1T_sb")
            h1n_all = bpool.tile([128, nw, 128], dt.float32, name="h1n_all")
            lg_all = bpool.tile([128, nw, 2], dt.float32, name="lg_all")

            w_start = np.concatenate([[0], np.cumsum(tiles_per_w)]).astype(int)

            def layer(src_dram, idx_sb, wl_sb, wr_sb, dense_rhs_sb, b_sb,
                      out_hT_sb, transpose_out):
                for w in range(nw):
                    t0, tw = int(w_start[w]), int(tiles_per_w[w])
                    # gather x[src] for this window's edges -> [128, tw, 128]
                    g = wpool.tile([128, tw, 128], dt.float32, tag="gather",
                                   bufs=3)
                    for t in range(tw):
                        nc.gpsimd.indirect_dma_start(
                            out=g[:, t, :],
                            out_offset=None,
                            in_=src_dram[:, :],
                            in_offset=bass.IndirectOffsetOnAxis(
                                ap=idx_sb[:, t0 + t:t0 + t + 1], axis=0),
                        )
                    # one-hot [128e, tw, 128d] = (iota[j] == dstoff[e,t])
                    oh = wpool.tile([128, tw, 128], dt.float32, tag="oh", bufs=3)
                    for t in range(tw):
                        nc.vector.tensor_scalar(
                            out=oh[:, t, :],
                            in0=iota_sb[:, :],
                            scalar1=dstoff_sb[:, t0 + t:t0 + t + 1],
                            scalar2=None,
                            op0=mybir.AluOpType.is_equal,
                        )
                    # segment sum: S[f, d] += G_t.T @ onehot_t
                    s_ps = ppool.tile([128, 128], dt.float32, tag="S", bufs=2)
                    for t in range(tw):
                        nc.tensor.matmul(
                            out=s_ps[:, :],
                            lhsT=g[:, t, :],
                            rhs=oh[:, t, :],
                            start=(t == 0),
                            stop=(t == tw - 1),
                        )
                    # mean: S * (1/deg), PSUM -> SBUF
                    s_sb = wpool.tile([128, 128], dt.float32, tag="ssb", bufs=2)
                    nc.vector.tensor_tensor(
                        out=s_sb[:, :], in0=s_ps[:, :],
                        in1=rdeg_sb[:, w * P:(w + 1) * P],
                        op=mybir.AluOpType.mult,
                    )
                    # dense: h = Wl @ S + Wr @ dense_rhs  (feature-major)
                    h_ps = ppool.tile([128, 128], dt.float32, tag="H", bufs=2)
                    nc.tensor.matmul(out=h_ps[:, :], lhsT=wl_sb[:, :],
                                     rhs=s_sb[:, :], start=True, stop=False)
                    nc.tensor.matmul(out=h_ps[:, :], lhsT=wr_sb[:, :],
                                     rhs=dense_rhs_sb[:, w * P:(w + 1) * P],
                                     start=False, stop=True)
                    # relu(h + b) -> out_hT[:, window]
                    nc.scalar.activation(
                        out=out_hT_sb[:, w * P:(w + 1) * P], in_=h_ps[:, :],
                        func=mybir.ActivationFunctionType.Relu,
                        bias=b_sb[:, :],
                    )
                    if transpose_out:
                        tr_ps = ppool.tile([128, 128], dt.float32, tag="TR",
                                           bufs=2)
                        nc.tensor.transpose(
                            out=tr_ps[:, :],
                            in_=out_hT_sb[:, w * P:(w + 1) * P],
                            identity=ident_sb[:, :],
                        )
                        nc.vector.tensor_copy(out=h1n_all[:, w, :],
                                              in_=tr_ps[:, :])
                    else:
                        # classification head on this window
                        h2T = out_hT_sb
                        lg_ps = ppool.tile([128, 2], dt.float32, tag="LG",
                                           bufs=2)
                        nc.tensor.matmul(out=lg_ps[:, :],
                                         lhsT=h2T[:, w * P:(w + 1) * P],
                                         rhs=wlinT_sb[:, :],
                                         start=True, stop=True)
                        nc.vector.tensor_tensor(
                            out=lg_all[:, w, :], in0=lg_ps[:, :],
                            in1=blin_sb[:, :], op=mybir.AluOpType.add)

            # ---- layer 1
            layer(x_full, idx1_sb, w_sb["W1lT"], w_sb["W1rT"], xT_sb, b1l_sb,
                  h1T_sb, transpose_out=True)
            # h1 (node-major) -> DRAM, then AllGather
            nc.sync.dma_start(
                h1_local[:, :].rearrange("(w p) f -> p w f", p=128),
                h1n_all[:, :, :],
            )
            nc.gpsimd.collective_compute(
                "AllGather",
                mybir.AluOpType.bypass,
                replica_groups=[list(range(n_cores))],
                ins=[h1_local[:, :].opt()],
                outs=[h1_full[:, :].opt()],
            )
            # ---- layer 2 (+ head)
            h2T_full = bpool.tile([128, nlp], dt.float32, name="h2T_full")
            layer(h1_full, idx2_sb, w_sb["W2lT"], w_sb["W2rT"], h1T_sb, b2l_sb,
                  h2T_full, transpose_out=False)

            # ---- log_softmax per window (per-partition-scalar ops only)
            m_all = bpool.tile([128, nw], dt.float32, name="m_all")
            nc.vector.tensor_reduce(out=m_all[:, :], in_=lg_all[:, :, :],
                                    axis=mybir.AxisListType.X,
                                    op=mybir.AluOpType.max)
            lm_all = bpool.tile([128, nw, 2], dt.float32, name="lm_all")
            e_all = bpool.tile([128, nw, 2], dt.float32, name="e_all")
            for w in range(nw):
                nc.vector.tensor_scalar(
                    out=lm_all[:, w, :], in0=lg_all[:, w, :],
                    scalar1=m_all[:, w:w + 1], scalar2=None,
                    op0=mybir.AluOpType.subtract)
                nc.scalar.activation(out=e_all[:, w, :], in_=lm_all[:, w, :],
                                     func=mybir.ActivationFunctionType.Exp)
            s_all = bpool.tile([128, nw], dt.float32, name="s_all")
            nc.vector.tensor_reduce(out=s_all[:, :], in_=e_all[:, :, :],
                                    axis=mybir.AxisListType.X,
                                    op=mybir.AluOpType.add)
            ls_all = bpool.tile([128, nw], dt.float32, name="ls_all")
            nc.scalar.activation(out=ls_all[:, :], in_=s_all[:, :],
                                 func=mybir.ActivationFunctionType.Ln)
            res_all = bpool.tile([128, nw, 2], dt.float32, name="res_all")
            for w in range(nw):
                nc.vector.tensor_scalar(
                    out=res_all[:, w, :], in0=lm_all[:, w, :],
                    scalar1=ls_all[:, w:w + 1], scalar2=None,
                    op0=mybir.AluOpType.subtract)
            nc.sync.dma_start(
                out_d[:, :].rearrange("(w p) c -> p w c", p=128),
                res_all[:, :, :],
            )

    nc.compile()
    return nc


# ---------------------------------------------------------------- entry point

def kernel(x, edge_index, W1l, b1l, W1r, W2l, b2l, W2r, Wlin, blin,
           _trace=False):
    from concourse.bass_utils import run_bass_kernel_spmd

    x = np.asarray(x, dtype=F32)
    idx1, idx2, dstoff, rdeg_core, tiles_per_w = _prep_graph(edge_index)

    nc = _build_nc(tiles_per_w, n_nodes=N_NODES, nl=NL, nw=NW)

    iota = np.broadcast_to(np.arange(128, dtype=F32), (128, 128)).copy()
    ident = np.eye(128, dtype=F32)
    common = {
        "x_full": x,
        "iota": iota,
        "ident": ident,
        "W1lT": np.ascontiguousarray(np.asarray(W1l, F32).T),
        "W1rT": np.ascontiguousarray(np.asarray(W1r, F32).T),
        "W2lT": np.ascontiguousarray(np.asarray(W2l, F32).T),
        "W2rT": np.ascontiguousarray(np.asarray(W2r, F32).T),
        "WlinT": np.ascontiguousarray(np.asarray(Wlin, F32).T),
        "b1l": np.asarray(b1l, F32).reshape(128, 1),
        "b2l": np.asarray(b2l, F32).reshape(128, 1),
        "blin": np.broadcast_to(np.asarray(blin, F32), (128, 2)).copy(),
    }
    in_maps = []
    for c in range(N_CORES):
        xc = x[c * NL:(c + 1) * NL]
        xT = np.zeros((128, NLP), dtype=F32)
        xT[:, :NL] = xc.T
        in_maps.append({
            **common,
            "xT": xT,
            "rdeg": rdeg_core[c],
            "idx1": idx1[c],
            "idx2": idx2[c],
            "dstoff": dstoff[c],
        })

    try:
        res = run_bass_kernel_spmd(nc, in_maps, core_ids=list(range(N_CORES)),
                                   trace=_trace)
    except ModuleNotFoundError:
        # axon NTFF profiling hook unavailable in this container
        res = run_bass_kernel_spmd(nc, in_maps, core_ids=list(range(N_CORES)),
                                   trace=False)
    out = np.concatenate([res.results[c]["out"][:NL] for c in range(N_CORES)],
                         axis=0)
    if _trace:
        kernel.last_results = res
    return out
